# revision 11
# baseline (speedup 1.0000x reference)
"""LxmertAttention cross-attention kernel for 8 Trainium2 NeuronCores.

Sharding: core c = b*2 + jh handles batch b and head-group jh (8 of 16 heads).
Host pre-work: transpose activations to [D, L] and weights to [D, Jh] (bf16),
fold the 1/sqrt(HD) scale into q_w, convert the attention mask into an
additive bias laid out per k-partition. Device work per core:
  Q^T = Wq^T.T @ Xh^T   (j on partitions, l free)     [512, 2048]
  K^T = Wk^T.T @ Xc^T                                  [512, 2048]
  V   = Xc^T.T @ Wv^T   (l on partitions, j free)      [2048, 512] (+ones col)
  per head h: S^T[k,q] = K_h^T.T @ Q_h^T (K=64 contraction)
              P^T = exp(S^T + mask_bias_k)  (ACT per-partition bias == masking)
              ctx[q, 0:64|denom] = P^T.T @ V''_h  (ones col gives denominator)
              out = ctx[:, 0:64] * recip(ctx[:, 64])
"""
import sys

sys.path.insert(0, "/opt/trn_rl_repo")

from contextlib import ExitStack

import ml_dtypes
import numpy as np

import concourse.bass as bass
import concourse.mybir as mybir
import concourse.tile as tile
from concourse import bacc
from concourse.bass_utils import run_bass_kernel_spmd

B, L, D, H, HD = 4, 2048, 1024, 16, 64
JH = D // 2          # 512 head-dims per core
NH = 8               # heads per core
NKT = L // 128       # 16 k tiles
BF = mybir.dt.bfloat16
F32 = mybir.dt.float32

PROFILE = False
LAST_RESULTS = None


def _emit(ctx, tc, xh, xc, wq, wk, wv, mb, out):
    nc = tc.nc
    consts = ctx.enter_context(tc.tile_pool(name="consts", bufs=1))
    ppool = ctx.enter_context(tc.tile_pool(name="pt", bufs=20))
    opool = ctx.enter_context(tc.tile_pool(name="osb", bufs=6))
    small = ctx.enter_context(tc.tile_pool(name="small", bufs=6))
    spool = ctx.enter_context(
        tc.tile_pool(name="spsum", bufs=2, space=bass.MemorySpace.PSUM)
    )
    cpool = ctx.enter_context(
        tc.tile_pool(name="cpsum", bufs=2, space=bass.MemorySpace.PSUM)
    )
    jpool = ctx.enter_context(
        tc.tile_pool(name="projpsum", bufs=2, space=bass.MemorySpace.PSUM)
    )

    # ---- load inputs ----
    xh_sb = consts.tile([128, 8, L], BF)   # Xh^T  [d_chunk partitions, chunk, l]
    xc_sb = consts.tile([128, 8, L], BF)
    wq_sb = consts.tile([128, 8, JH], BF)
    wk_sb = consts.tile([128, 8, JH], BF)
    wv_sb = consts.tile([128, 8, JH], BF)
    mb_sb = consts.tile([128, NKT], F32)
    xh_c = xh.rearrange("(c p) l -> c p l", p=128)
    xc_c = xc.rearrange("(c p) l -> c p l", p=128)
    wq_c = wq.rearrange("(c p) j -> c p j", p=128)
    wk_c = wk.rearrange("(c p) j -> c p j", p=128)
    wv_c = wv.rearrange("(c p) j -> c p j", p=128)
    for c in range(8):
        nc.sync.dma_start(xh_sb[:, c, :], xh_c[c])
        nc.sync.dma_start(xc_sb[:, c, :], xc_c[c])
        nc.sync.dma_start(wq_sb[:, c, :], wq_c[c])
        nc.sync.dma_start(wk_sb[:, c, :], wk_c[c])
        nc.sync.dma_start(wv_sb[:, c, :], wv_c[c])
    nc.sync.dma_start(mb_sb, mb)

    qt_sb = consts.tile([128, 4, L], BF)   # Q^T per head-pair
    kt_sb = consts.tile([128, 4, L], BF)
    vpp = consts.tile([128, NKT, NH * 65], BF)  # V per k tile, 8x(64 dims + ones)

    # ---- V projection: out[l, j] with ones column per head ----
    # (emitted after pair-0 Q/K proj: V is first needed only at the ctx
    # stage, so this PE work hides under the first head's exp phase)
    def v_proj():
        for lt in range(NKT):
            ps = jpool.tile([128, JH], F32)
            for dc in range(8):
                nc.tensor.matmul(
                    ps,
                    xc_sb[:, dc, lt * 128:(lt + 1) * 128],
                    wv_sb[:, dc, :],
                    start=(dc == 0),
                    stop=(dc == 7),
                )
            vh = vpp[:, lt, :].rearrange("p (h e) -> p h e", e=65)
            nc.vector.tensor_copy(
                vh[:, :, 0:64], ps.rearrange("p (h d) -> p h d", d=64)
            )
            nc.vector.memset(vh[:, :, 64:65], 1.0)

    def qk_proj(pair):
        for (w_sb, x_sb, dst) in ((wq_sb, xh_sb, qt_sb), (wk_sb, xc_sb, kt_sb)):
            for lg in range(4):
                ps = jpool.tile([128, 512], F32)
                for dc in range(8):
                    nc.tensor.matmul(
                        ps,
                        w_sb[:, dc, pair * 128:(pair + 1) * 128],
                        x_sb[:, dc, lg * 512:(lg + 1) * 512],
                        start=(dc == 0),
                        stop=(dc == 7),
                    )
                nc.vector.tensor_copy(dst[:, pair, lg * 512:(lg + 1) * 512], ps)

    def attention(pair):
        for qb in range(2):
            q0 = qb * 1024
            for hp in range(2):
                h = pair * 2 + hp
                b0 = hp * 64
                pts = []
                for kt in range(NKT):
                    s_ps = spool.tile([128, 1024], F32)
                    for qh in range(2):
                        nc.tensor.matmul(
                            s_ps[:, qh * 512:(qh + 1) * 512],
                            kt_sb[b0:b0 + 64, pair, kt * 128:(kt + 1) * 128],
                            qt_sb[b0:b0 + 64, pair, q0 + qh * 512:q0 + (qh + 1) * 512],
                            start=True,
                            stop=True,
                        )
                    pt = ppool.tile([128, 1024], BF)
                    nc.scalar.activation(
                        pt,
                        s_ps,
                        mybir.ActivationFunctionType.Exp,
                        bias=mb_sb[:, kt:kt + 1],
                    )
                    pts.append(pt)
                for qt in range(8):
                    c_ps = cpool.tile([128, 65], F32)
                    for kt in range(NKT):
                        nc.tensor.matmul(
                            c_ps,
                            pts[kt][:, qt * 128:(qt + 1) * 128],
                            vpp[:, kt, h * 65:(h + 1) * 65],
                            start=(kt == 0),
                            stop=(kt == NKT - 1),
                        )
                    recip = small.tile([128, 1], F32)
                    nc.vector.reciprocal(recip, c_ps[:, 64:65])
                    o_sb = opool.tile([128, 64], F32)
                    nc.vector.tensor_scalar_mul(o_sb, c_ps[:, 0:64], recip)
                    nc.sync.dma_start(
                        out[q0 + qt * 128:q0 + (qt + 1) * 128, h * 64:(h + 1) * 64],
                        o_sb,
                    )

    v_proj()
    for pair in range(4):
        qk_proj(pair)
        attention(pair)


def _build_program():
    nc = bacc.Bacc("TRN2", target_bir_lowering=False, debug=False)
    xh = nc.dram_tensor("xh_t", (D, L), BF, kind="ExternalInput")
    xc = nc.dram_tensor("xc_t", (D, L), BF, kind="ExternalInput")
    wq = nc.dram_tensor("wq_t", (D, JH), BF, kind="ExternalInput")
    wk = nc.dram_tensor("wk_t", (D, JH), BF, kind="ExternalInput")
    wv = nc.dram_tensor("wv_t", (D, JH), BF, kind="ExternalInput")
    mb = nc.dram_tensor("mb", (128, NKT), F32, kind="ExternalInput")
    out = nc.dram_tensor("out", (L, JH), F32, kind="ExternalOutput")
    with tile.TileContext(nc) as tc, ExitStack() as ctx:
        _emit(ctx, tc, xh.ap(), xc.ap(), wq.ap(), wk.ap(), wv.ap(), mb.ap(), out.ap())
    nc.compile()
    return nc


_CACHE = {}


def _get_program():
    if "nc" not in _CACHE:
        _CACHE["nc"] = _build_program()
    return _CACHE["nc"]


def kernel(hidden_states, context, attention_mask, q_w, q_b, k_w, k_b, v_w, v_b):
    global LAST_RESULTS
    nc = _get_program()
    bf = ml_dtypes.bfloat16

    hidden_states = np.asarray(hidden_states, np.float32)
    context = np.asarray(context, np.float32)
    attention_mask = np.asarray(attention_mask)

    wts = {}
    for name, w in (("wq_t", np.asarray(q_w, np.float32) / 8.0),
                    ("wk_t", np.asarray(k_w, np.float32)),
                    ("wv_t", np.asarray(v_w, np.float32))):
        wts[name] = [
            np.ascontiguousarray(w[j * JH:(j + 1) * JH, :].T).astype(bf)
            for j in range(2)
        ]

    in_maps = []
    for c in range(8):
        b, jh = c // 2, c % 2
        xh = np.ascontiguousarray(hidden_states[b].T).astype(bf)
        xc = np.ascontiguousarray(context[b].T).astype(bf)
        mbias = np.where(attention_mask[b] > 0, np.float32(-1e9), np.float32(0.0))
        mbias = np.ascontiguousarray(mbias.astype(np.float32).reshape(NKT, 128).T)
        in_maps.append({
            "xh_t": xh, "xc_t": xc,
            "wq_t": wts["wq_t"][jh], "wk_t": wts["wk_t"][jh],
            "wv_t": wts["wv_t"][jh], "mb": mbias,
        })

    res = run_bass_kernel_spmd(nc, in_maps, core_ids=list(range(8)), trace=PROFILE)
    LAST_RESULTS = res

    out = np.empty((B, L, D), np.float32)
    for c in range(8):
        b, jh = c // 2, c % 2
        out[b, :, jh * JH:(jh + 1) * JH] = res.results[c]["out"]
    return out
